# revision 7
# baseline (speedup 1.0000x reference)
"""Llama GQA attention layer (B=1, S=2048, D=2048, H=32, KVH=8, HD=64), fp32,
tensor-parallel over 8 trn2 NeuronCores: heads sharded 4 q-heads + 1 kv-head
per core, all-gather of attention outputs, Wo column-sharded.

Self-contained: hardcodes shapes; uses the concourse Bass/Tile stack from
/opt/trn_rl_repo.
"""

import math
import os
import sys
import tempfile

import numpy as np

sys.path.insert(0, "/opt/trn_rl_repo")

from concourse import bacc, mybir, tile  # noqa: E402
from concourse.bass_utils import run_bass_kernel_spmd  # noqa: E402

F32 = mybir.dt.float32
F32R = mybir.dt.float32r

P = 128
D = 2048
KO = D // P          # 16 contraction chunks
H = 32
KVH = 8
HD = 64
NCORES = 8
HPC = H // NCORES    # 4 query heads per core
MQ = HPC * HD // P   # 2 x 128 chunks of per-core q dim (256)
THETA = 10000.0
NEG = -1.0e30

# Precision knobs: float32r streams 1 col/cycle (vs 4 for fp32) when the
# matmul free dim >= 256.  Env-overridable so test.py can A/B without edits.
def _flag(name, default):
    v = os.environ.get(name)
    return default if v is None else v not in ("0", "false", "")

F32R_QKV = _flag("ATTN_F32R_QKV", False)
F32R_SCORES = _flag("ATTN_F32R_SCORES", True)
F32R_TRANS = _flag("ATTN_F32R_TRANS", True)
F32R_PV = _flag("ATTN_F32R_PV", True)
F32R_WO = _flag("ATTN_F32R_WO", False)


def _mm(nc, out, lhsT, rhs, start, stop, f32r):
    if f32r:
        lhsT = lhsT.bitcast(F32R)
        rhs = rhs.bitcast(F32R)
    nc.tensor.matmul(out, lhsT, rhs, start=start, stop=stop)


def _consts(S):
    i = np.arange(32, dtype=np.float64)
    inv_freq = 1.0 / (THETA ** (2.0 * i / HD))
    t = np.arange(S, dtype=np.float64)
    freqs = np.outer(t, inv_freq)            # [S, 32]
    cos = np.cos(freqs).astype(np.float32)
    sin = np.sin(freqs).astype(np.float32)
    rows = (np.arange(P) % HD) // 2          # pair index per partition row
    cosf = cos[:, rows].T.copy()             # [128, S]
    sinf = sin[:, rows].T.copy()
    # rot = R @ x with rot[2i] = -x[2i+1], rot[2i+1] = x[2i]; matmul computes
    # lhsT.T @ rhs, so pass R.T.
    R = np.zeros((P, P), np.float32)
    ii = np.arange(0, P, 2)
    R[ii, ii + 1] = -1.0
    R[ii + 1, ii] = 1.0
    ident = np.eye(P, dtype=np.float32)
    # in-tile causal mask for the diagonal 128x128 block: 0 on/below diag
    q = np.arange(P)[:, None]
    k = np.arange(P)[None, :]
    maskneg = np.where(k <= q, 0.0, NEG).astype(np.float32)
    return cosf, sinf, np.ascontiguousarray(R.T), ident, maskneg


def build_program(S):
    SCW = 256            # s-chunk width for the qkv phase
    NSC = S // SCW
    NQB = S // 512       # 512-wide query blocks in attention
    nc = bacc.Bacc("TRN2", target_bir_lowering=False, debug=False,
                   enable_asserts=False, num_devices=NCORES)

    xT_d = nc.dram_tensor("xT", [P, KO, S], F32, kind="ExternalInput")
    wq_d = nc.dram_tensor("wq", [P, KO, MQ * P], F32, kind="ExternalInput")
    wk_d = nc.dram_tensor("wk", [P, KO, HD], F32, kind="ExternalInput")
    wv_d = nc.dram_tensor("wv", [P, KO, HD], F32, kind="ExternalInput")
    wo_d = nc.dram_tensor("wo", [P, KO, MQ * P], F32, kind="ExternalInput")
    outT_d = nc.dram_tensor("outT", [MQ, P, S], F32, kind="ExternalOutput")

    cosf_np, sinf_np, rotT_np, ident_np, mask_np = _consts(S)
    cosf_d = nc.inline_tensor(cosf_np, name="cosf")
    sinf_d = nc.inline_tensor(sinf_np, name="sinf")
    rotT_d = nc.inline_tensor(rotT_np, name="rotT")
    ident_d = nc.inline_tensor(ident_np, name="ident")
    mask_d = nc.inline_tensor(mask_np, name="maskneg")

    Exp = mybir.ActivationFunctionType.Exp

    with tile.TileContext(nc) as tc:
        with (
            tc.tile_pool(name="consts", bufs=1) as cp,
            tc.tile_pool(name="persist", bufs=1) as pp,
            tc.tile_pool(name="dram", bufs=1, space="DRAM") as dp,
        ):
            wq_sb = cp.tile([P, KO, MQ * P], F32)
            nc.sync.dma_start(wq_sb, wq_d[:])
            wk_sb = cp.tile([P, KO, HD], F32)
            nc.sync.dma_start(wk_sb, wk_d[:])
            wv_sb = cp.tile([P, KO, HD], F32)
            nc.sync.dma_start(wv_sb, wv_d[:])
            wo_sb = cp.tile([P, KO, MQ * P], F32)
            nc.sync.dma_start(wo_sb, wo_d[:])
            rotT_sb = cp.tile([P, P], F32)
            nc.sync.dma_start(rotT_sb, rotT_d[:])
            ident_sb = cp.tile([P, P], F32)
            nc.sync.dma_start(ident_sb, ident_d[:])
            mask_sb = cp.tile([P, P], F32)
            nc.sync.dma_start(mask_sb, mask_d[:])
            ident_r = cp.tile([P, P], F32R if F32R_TRANS else F32)
            nc.vector.tensor_copy(ident_r, ident_sb)

            # persistent activations
            qTa = pp.tile([HD, HPC, S], F32R if F32R_SCORES else F32)      # roped q, head-major
            kTr = pp.tile([HD, S], F32R if F32R_SCORES else F32)           # roped k
            v_sb = pp.tile([P, S // P, HD], F32R if F32R_PV else F32)  # v, [s-tile, s-in, hd]
            attnT = pp.tile([HD, HPC, S], F32)    # attention out, transposed

            # ---------------- Phase A: QKV projections + RoPE ------------
            with (
                tc.tile_pool(name="cs", bufs=1) as csp,
                tc.tile_pool(name="xin", bufs=2) as xin,
                tc.tile_pool(name="rtmp", bufs=2) as rtmp,
                tc.tile_pool(name="psA", bufs=2, space="PSUM") as psA,
            ):
                cosf_sb = csp.tile([P, S], F32)
                nc.sync.dma_start(cosf_sb, cosf_d[:])
                sinf_sb = csp.tile([P, S], F32)
                nc.sync.dma_start(sinf_sb, sinf_d[:])

                for sc in range(NSC):
                    sl = slice(sc * SCW, (sc + 1) * SCW)
                    xt = xin.tile([P, KO, SCW], F32, tag="xt")
                    nc.sync.dma_start(xt, xT_d[:, :, sl])
                    # q projections: two 128-row chunks of qT (2 heads each)
                    for m2 in range(MQ):
                        qp = psA.tile([P, SCW], F32, tag="qp")
                        for ko in range(KO):
                            _mm(nc, qp, wq_sb[:, ko, m2 * P:(m2 + 1) * P],
                                xt[:, ko], ko == 0, ko == KO - 1, F32R_QKV)
                        qs = rtmp.tile([P, SCW], F32, tag="qs")
                        nc.scalar.copy(qs, qp)
                        rp = psA.tile([P, SCW], F32, tag="rp")
                        _mm(nc, rp, rotT_sb, qs, True, True, False)
                        t0 = rtmp.tile([P, SCW], F32, tag="t0")
                        nc.vector.tensor_mul(t0, qs, cosf_sb[:, sl])
                        t1 = rtmp.tile([P, SCW], F32, tag="t1")
                        nc.vector.tensor_mul(t1, rp, sinf_sb[:, sl])
                        tq = rtmp.tile([P, SCW], F32, tag="tq")
                        nc.vector.tensor_add(tq, t0, t1)
                        # head-major scatter; compute-engine copies round
                        # to f32r (DMA cannot), upper half shifts via staging
                        nc.vector.tensor_copy(qTa[:, 2 * m2, sl], tq[0:HD])
                        stg = rtmp.tile([HD, SCW], F32, tag="stg")
                        nc.sync.dma_start(stg, tq[HD:P])
                        nc.vector.tensor_copy(qTa[:, 2 * m2 + 1, sl], stg)
                    # k projection (single kv head)
                    kp = psA.tile([HD, SCW], F32, tag="qp")
                    for ko in range(KO):
                        _mm(nc, kp, wk_sb[:, ko], xt[:, ko],
                            ko == 0, ko == KO - 1, F32R_QKV)
                    ks = rtmp.tile([HD, SCW], F32, tag="qs")
                    nc.scalar.copy(ks, kp)
                    rpk = psA.tile([HD, SCW], F32, tag="rp")
                    _mm(nc, rpk, rotT_sb[0:HD, 0:HD], ks, True, True, False)
                    t0k = rtmp.tile([HD, SCW], F32, tag="t0")
                    nc.vector.tensor_mul(t0k, ks, cosf_sb[0:HD, sl])
                    t1k = rtmp.tile([HD, SCW], F32, tag="t1")
                    nc.vector.tensor_mul(t1k, rpk, sinf_sb[0:HD, sl])
                    nc.vector.tensor_add(kTr[:, sl], t0k, t1k)
                    # v projection: [s, hd] layout (s on partitions)
                    for si in range(SCW // P):
                        vp = psA.tile([P, HD], F32, tag="vp")
                        for ko in range(KO):
                            _mm(nc, vp, xt[:, ko, si * P:(si + 1) * P],
                                wv_sb[:, ko], ko == 0, ko == KO - 1, F32R_QKV)
                        nc.scalar.copy(v_sb[:, sc * (SCW // P) + si], vp)

            # ---------------- Phase B: causal GQA attention --------------
            with (
                tc.tile_pool(name="probs", bufs=2) as prp,
                tc.tile_pool(name="sums", bufs=3) as smp,
                tc.tile_pool(name="ptb", bufs=1) as ptp,
                tc.tile_pool(name="psB", bufs=1, space="PSUM") as psB,
            ):
                for h in range(HPC):
                    for qb in range(NQB):
                        nkt = 4 * (qb + 1)
                        pts = ptp.tile([P, S // P, 512], F32R if F32R_PV else F32,
                                       tag="pts")
                        for r in range(1, 4):
                            nc.vector.memzero(pts[:, qb * 4 + r, 0:r * P])
                        for ql in range(4):
                            qi = qb * 4 + ql
                            nk = qi + 1
                            nch = (nk + 3) // 4
                            pr = prp.tile([P, S], F32R if F32R_TRANS else F32,
                                          tag="pr")
                            acc = smp.tile([P, 4], F32, tag="acc")
                            for c in range(nch):
                                w = min(512, nk * P - c * 512)
                                sp = psB.tile([P, 512], F32, tag="sp", bufs=3)
                                _mm(nc, sp[:, :w],
                                    qTa[:, h, qi * P:(qi + 1) * P],
                                    kTr[:, c * 512:c * 512 + w],
                                    True, True, F32R_SCORES)
                                if c == nch - 1:
                                    nc.vector.tensor_add(
                                        sp[:, w - P:w], sp[:, w - P:w], mask_sb)
                                nc.scalar.activation(
                                    pr[:, c * 512:c * 512 + w], sp[:, :w], Exp,
                                    scale=1.0 / math.sqrt(HD),
                                    accum_out=acc[:, c:c + 1])
                            tot = smp.tile([P, 1], F32, tag="tot")
                            nc.vector.tensor_reduce(
                                tot, acc[:, 0:nch], axis=mybir.AxisListType.X,
                                op=mybir.AluOpType.add)
                            rcp = smp.tile([P, 1], F32, tag="rcp")
                            nc.vector.reciprocal(rcp, tot)
                            nc.vector.tensor_scalar_mul(
                                pr[:, :nk * P], pr[:, :nk * P], rcp)
                            for kt in range(nk):
                                tp = psB.tile([P, P], F32R if F32R_TRANS else F32,
                                              tag="tp", bufs=2)
                                nc.tensor.transpose(
                                    tp, pr[:, kt * P:(kt + 1) * P], ident_r)
                                nc.any.tensor_copy(
                                    pts[:, kt, ql * P:(ql + 1) * P], tp)
                        op = psB.tile([HD, 512], F32, tag="op", bufs=2)
                        for kt in range(nkt):
                            _mm(nc, op, v_sb[:, kt], pts[:, kt],
                                kt == 0, kt == nkt - 1, F32R_PV)
                        nc.any.tensor_copy(
                            attnT[:, h, qb * 512:(qb + 1) * 512], op)

            # ---------------- Phase C: all-gather + Wo -------------------
            agin = dp.tile([HPC * HD, S], F32)
            agout = dp.tile([H * HD, S], F32, addr_space="Shared")
            for h in range(HPC):
                nc.sync.dma_start(agin[h * HD:(h + 1) * HD], attnT[:, h])
            nc.gpsimd.collective_compute(
                "AllGather", mybir.AluOpType.bypass,
                replica_groups=[list(range(NCORES))],
                ins=[agin.opt()], outs=[agout.opt()])

            with (
                tc.tile_pool(name="rhp", bufs=3) as rhp,
                tc.tile_pool(name="osb", bufs=2) as osb,
                tc.tile_pool(name="psC", bufs=2, space="PSUM") as psC,
            ):
                FO = H * HD // P  # 16 feature chunks
                for sc in range(S // 512):
                    sl = slice(sc * 512, (sc + 1) * 512)
                    wop = [psC.tile([P, 512], F32, tag=f"wop{c2}",
                                    name=f"wop{c2}_{sc}")
                           for c2 in range(MQ)]
                    for fo in range(FO):
                        rt = rhp.tile([P, 512], F32, tag="rt")
                        nc.sync.dma_start(
                            rt, agout[fo * P:(fo + 1) * P, sl])
                        for c2 in range(MQ):
                            _mm(nc, wop[c2], wo_sb[:, fo, c2 * P:(c2 + 1) * P],
                                rt, fo == 0, fo == FO - 1, F32R_WO)
                    for c2 in range(MQ):
                        os_t = osb.tile([P, 512], F32, tag="os")
                        nc.scalar.copy(os_t, wop[c2])
                        nc.sync.dma_start(outT_d[c2, :, sl], os_t)

    nc.compile()
    return nc


def make_in_maps(hidden_states, Wq, Wk, Wv, Wo):
    S = hidden_states.shape[1]
    xT = np.ascontiguousarray(hidden_states.reshape(S, D).T)  # [D, S]
    xTd = np.ascontiguousarray(
        xT.reshape(KO, P, S).transpose(1, 0, 2))              # [128, 16, S]

    def wslice(w, j, width):
        ws = w[:, j * width:(j + 1) * width]                  # [D, width]
        return np.ascontiguousarray(ws.reshape(KO, P, width).transpose(1, 0, 2))

    in_maps = []
    for j in range(NCORES):
        in_maps.append({
            "xT": xTd,
            "wq": wslice(Wq, j, MQ * P),
            "wk": wslice(Wk, j, HD),
            "wv": wslice(Wv, j, HD),
            "wo": wslice(Wo, j, MQ * P),
        })
    return in_maps


def assemble_output(results, S):
    out = np.empty((S, D), np.float32)
    for j in range(NCORES):
        blk = results[j]["outT"].reshape(MQ * P, S)           # [256, S]
        out[:, j * MQ * P:(j + 1) * MQ * P] = blk.T
    return out.reshape(1, S, D)


_PROG = {}
_RUN = {}
LAST = {}


class _Runner:
    """Persistent jitted SPMD executor mirroring bass2jax.run_bass_via_pjrt,
    kept alive so repeated calls skip retracing and input re-transfer."""

    def __init__(self, nc, n_cores):
        import jax
        from jax.experimental.shard_map import shard_map
        from jax.sharding import Mesh, NamedSharding, PartitionSpec

        from concourse import mybir as _mb
        from concourse.bass2jax import (_bass_exec_p, install_neuronx_cc_hook,
                                        partition_id_tensor)

        install_neuronx_cc_hook()
        self.jax = jax
        self.n_cores = n_cores
        partition_name = (nc.partition_id_tensor.name
                          if nc.partition_id_tensor else None)
        in_names, out_names, out_avals, zero_outs = [], [], [], []
        for alloc in nc.m.functions[0].allocations:
            if not isinstance(alloc, _mb.MemoryLocationSet):
                continue
            name = alloc.memorylocations[0].name
            if alloc.kind == "ExternalInput":
                if name != partition_name:
                    in_names.append(name)
            elif alloc.kind == "ExternalOutput":
                out_names.append(name)
                shape = tuple(alloc.tensor_shape)
                dtype = _mb.dt.np(alloc.dtype)
                out_avals.append(jax.core.ShapedArray(shape, dtype))
                zero_outs.append(np.zeros(shape, dtype))
        n_params = len(in_names)
        self.in_names = list(in_names)
        self.out_names = out_names
        self.out_avals = out_avals
        self.zero_outs = zero_outs
        all_in_names = in_names + out_names
        if partition_name is not None:
            all_in_names.append(partition_name)

        def _body(*args):
            operands = list(args)
            if partition_name is not None:
                operands.append(partition_id_tensor())
            outs = _bass_exec_p.bind(
                *operands, out_avals=tuple(out_avals),
                in_names=tuple(all_in_names), out_names=tuple(out_names),
                lowering_input_output_aliases=(), sim_require_finite=True,
                sim_require_nnan=True, nc=nc)
            return tuple(outs)

        devices = jax.devices()[:n_cores]
        self.mesh = Mesh(np.asarray(devices), ("core",))
        self.sharding = NamedSharding(self.mesh, PartitionSpec("core"))
        n_outs = len(out_names)
        self.fn = jax.jit(
            shard_map(_body, mesh=self.mesh,
                      in_specs=(PartitionSpec("core"),) * (n_params + n_outs),
                      out_specs=(PartitionSpec("core"),) * n_outs,
                      check_rep=False),
            donate_argnums=tuple(range(n_params, n_params + n_outs)),
            keep_unused=True)

    def put_inputs(self, in_maps):
        cat = [np.concatenate([np.asarray(m[n]) for m in in_maps], axis=0)
               for n in self.in_names]
        return [self.jax.device_put(a, self.sharding) for a in cat]

    def put_zeros(self):
        return [self.jax.device_put(
                    np.zeros((self.n_cores * z.shape[0], *z.shape[1:]), z.dtype),
                    self.sharding)
                for z in self.zero_outs]

    def run(self, dev_in, dev_zeros):
        outs = self.fn(*dev_in, *dev_zeros)
        self.jax.block_until_ready(outs)
        return outs

    def results(self, outs):
        return [
            {name: np.asarray(outs[i]).reshape(self.n_cores,
                                               *self.out_avals[i].shape)[c]
             for i, name in enumerate(self.out_names)}
            for c in range(self.n_cores)
        ]


def _get_runner(S):
    if S not in _RUN:
        if S not in _PROG:
            _PROG[S] = build_program(S)
        _RUN[S] = _Runner(_PROG[S], NCORES)
    return _RUN[S]


def kernel(hidden_states, Wq, Wk, Wv, Wo):
    import time
    hidden_states = np.asarray(hidden_states, np.float32)
    S = hidden_states.shape[1]
    r = _get_runner(S)
    in_maps = make_in_maps(hidden_states, np.asarray(Wq, np.float32),
                           np.asarray(Wk, np.float32),
                           np.asarray(Wv, np.float32),
                           np.asarray(Wo, np.float32))
    dev_in = r.put_inputs(in_maps)
    outs = r.run(dev_in, r.put_zeros())
    results = r.results(outs)
    if os.environ.get("ATTN_BENCH", "0") != "0":
        times = []
        for _ in range(int(os.environ.get("ATTN_BENCH_ITERS", "6"))):
            zs = r.put_zeros()
            t0 = time.perf_counter()
            o = r.run(dev_in, zs)
            times.append(time.perf_counter() - t0)
            del o
        LAST["bench_times"] = times
        LAST["exec_time_ns"] = int(min(times) * 1e9)
    return assemble_output(results, S)


# revision 9
# speedup vs baseline: 1.2624x; 1.2624x over previous
"""Llama GQA attention layer (B=1, S=2048, D=2048, H=32, KVH=8, HD=64), fp32,
tensor-parallel over 8 trn2 NeuronCores: heads sharded 4 q-heads + 1 kv-head
per core, all-gather of attention outputs, Wo column-sharded.

Self-contained: hardcodes shapes; uses the concourse Bass/Tile stack from
/opt/trn_rl_repo.
"""

import math
import os
import sys
import tempfile

import numpy as np

sys.path.insert(0, "/opt/trn_rl_repo")

from concourse import bacc, mybir, tile  # noqa: E402
from concourse.bass_utils import run_bass_kernel_spmd  # noqa: E402

F32 = mybir.dt.float32
F32R = mybir.dt.float32r

P = 128
D = 2048
KO = D // P          # 16 contraction chunks
H = 32
KVH = 8
HD = 64
NCORES = 8
HPC = H // NCORES    # 4 query heads per core
MQ = HPC * HD // P   # 2 x 128 chunks of per-core q dim (256)
THETA = 10000.0
NEG = -1.0e30

# Precision knobs: float32r streams 1 col/cycle (vs 4 for fp32) when the
# matmul free dim >= 256.  Env-overridable so test.py can A/B without edits.
def _flag(name, default):
    v = os.environ.get(name)
    return default if v is None else v not in ("0", "false", "")

F32R_QKV = _flag("ATTN_F32R_QKV", False)
F32R_SCORES = _flag("ATTN_F32R_SCORES", True)
F32R_TRANS = _flag("ATTN_F32R_TRANS", True)
F32R_PV = _flag("ATTN_F32R_PV", True)
F32R_WO = _flag("ATTN_F32R_WO", False)


def _mm(nc, out, lhsT, rhs, start, stop, f32r):
    if f32r:
        lhsT = lhsT.bitcast(F32R)
        rhs = rhs.bitcast(F32R)
    nc.tensor.matmul(out, lhsT, rhs, start=start, stop=stop)


def _consts(S):
    i = np.arange(32, dtype=np.float64)
    inv_freq = 1.0 / (THETA ** (2.0 * i / HD))
    t = np.arange(S, dtype=np.float64)
    freqs = np.outer(t, inv_freq)            # [S, 32]
    cos = np.cos(freqs).astype(np.float32)
    sin = np.sin(freqs).astype(np.float32)
    rows = (np.arange(P) % HD) // 2          # pair index per partition row
    cosf = cos[:, rows].T.copy()             # [128, S]
    sinf = sin[:, rows].T.copy()
    # rot = R @ x with rot[2i] = -x[2i+1], rot[2i+1] = x[2i]; matmul computes
    # lhsT.T @ rhs, so pass R.T.
    R = np.zeros((P, P), np.float32)
    ii = np.arange(0, P, 2)
    R[ii, ii + 1] = -1.0
    R[ii + 1, ii] = 1.0
    ident = np.eye(P, dtype=np.float32)
    # in-tile causal mask for the diagonal 128x128 block: 0 on/below diag
    q = np.arange(P)[:, None]
    k = np.arange(P)[None, :]
    maskneg = np.where(k <= q, 0.0, NEG).astype(np.float32)
    return cosf, sinf, np.ascontiguousarray(R.T), ident, maskneg


def build_program(S):
    SCW = 256            # s-chunk width for the qkv phase
    NSC = S // SCW
    NQB = S // 512       # 512-wide query blocks in attention
    nc = bacc.Bacc("TRN2", target_bir_lowering=False, debug=False,
                   enable_asserts=False, num_devices=NCORES)

    xT_d = nc.dram_tensor("xT", [P, KO, S], F32, kind="ExternalInput")
    wq_d = nc.dram_tensor("wq", [P, KO, MQ * P], F32, kind="ExternalInput")
    wk_d = nc.dram_tensor("wk", [P, KO, HD], F32, kind="ExternalInput")
    wv_d = nc.dram_tensor("wv", [P, KO, HD], F32, kind="ExternalInput")
    wo_d = nc.dram_tensor("wo", [P, KO, MQ * P], F32, kind="ExternalInput")
    outT_d = nc.dram_tensor("outT", [MQ, P, S], F32, kind="ExternalOutput")

    cosf_np, sinf_np, rotT_np, ident_np, mask_np = _consts(S)
    cosf_d = nc.inline_tensor(cosf_np, name="cosf")
    sinf_d = nc.inline_tensor(sinf_np, name="sinf")
    rotT_d = nc.inline_tensor(rotT_np, name="rotT")
    ident_d = nc.inline_tensor(ident_np, name="ident")
    mask_d = nc.inline_tensor(mask_np, name="maskneg")

    Exp = mybir.ActivationFunctionType.Exp

    with tile.TileContext(nc) as tc:
        with (
            tc.tile_pool(name="consts", bufs=1) as cp,
            tc.tile_pool(name="persist", bufs=1) as pp,
            tc.tile_pool(name="dram", bufs=1, space="DRAM") as dp,
        ):
            wq_sb = cp.tile([P, KO, MQ * P], F32)
            nc.sync.dma_start(wq_sb, wq_d[:])
            wk_sb = cp.tile([P, KO, HD], F32)
            nc.sync.dma_start(wk_sb, wk_d[:])
            wv_sb = cp.tile([P, KO, HD], F32)
            nc.sync.dma_start(wv_sb, wv_d[:])
            wo_sb = cp.tile([P, KO, MQ * P], F32)
            nc.sync.dma_start(wo_sb, wo_d[:])
            rotT_sb = cp.tile([P, P], F32)
            nc.sync.dma_start(rotT_sb, rotT_d[:])
            ident_sb = cp.tile([P, P], F32)
            nc.sync.dma_start(ident_sb, ident_d[:])
            mask_sb = cp.tile([P, P], F32)
            nc.sync.dma_start(mask_sb, mask_d[:])
            ident_r = cp.tile([P, P], F32R if F32R_TRANS else F32)
            nc.vector.tensor_copy(ident_r, ident_sb)

            # persistent activations
            qTa = pp.tile([HD, HPC, S], F32R if F32R_SCORES else F32)      # roped q, head-major
            kTr = pp.tile([HD, S], F32R if F32R_SCORES else F32)           # roped k
            v_sb = pp.tile([P, S // P, HD], F32R if F32R_PV else F32)  # v, [s-tile, s-in, hd]
            attnT = pp.tile([HD, HPC, S], F32)    # attention out, transposed

            # ---------------- Phase A: QKV projections + RoPE ------------
            with (
                tc.tile_pool(name="cs", bufs=1) as csp,
                tc.tile_pool(name="xin", bufs=2) as xin,
                tc.tile_pool(name="rtmp", bufs=2) as rtmp,
                tc.tile_pool(name="psA", bufs=2, space="PSUM") as psA,
            ):
                cosf_sb = csp.tile([P, S], F32)
                nc.sync.dma_start(cosf_sb, cosf_d[:])
                sinf_sb = csp.tile([P, S], F32)
                nc.sync.dma_start(sinf_sb, sinf_d[:])

                for sc in range(NSC):
                    sl = slice(sc * SCW, (sc + 1) * SCW)
                    xt = xin.tile([P, KO, SCW], F32, tag="xt")
                    nc.sync.dma_start(xt, xT_d[:, :, sl])
                    # q projections: two 128-row chunks of qT (2 heads each)
                    for m2 in range(MQ):
                        qp = psA.tile([P, SCW], F32, tag="qp")
                        for ko in range(KO):
                            _mm(nc, qp, wq_sb[:, ko, m2 * P:(m2 + 1) * P],
                                xt[:, ko], ko == 0, ko == KO - 1, F32R_QKV)
                        qs = rtmp.tile([P, SCW], F32, tag="qs")
                        nc.scalar.copy(qs, qp)
                        rp = psA.tile([P, SCW], F32, tag="rp")
                        _mm(nc, rp, rotT_sb, qs, True, True, False)
                        t0 = rtmp.tile([P, SCW], F32, tag="t0")
                        nc.vector.tensor_mul(t0, qs, cosf_sb[:, sl])
                        t1 = rtmp.tile([P, SCW], F32, tag="t1")
                        nc.vector.tensor_mul(t1, rp, sinf_sb[:, sl])
                        tq = rtmp.tile([P, SCW], F32, tag="tq")
                        nc.vector.tensor_add(tq, t0, t1)
                        # head-major scatter; compute-engine copies round
                        # to f32r (DMA cannot), upper half shifts via staging
                        nc.vector.tensor_copy(qTa[:, 2 * m2, sl], tq[0:HD])
                        stg = rtmp.tile([HD, SCW], F32, tag="stg")
                        nc.sync.dma_start(stg, tq[HD:P])
                        nc.vector.tensor_copy(qTa[:, 2 * m2 + 1, sl], stg)
                    # k projection (single kv head)
                    kp = psA.tile([HD, SCW], F32, tag="qp")
                    for ko in range(KO):
                        _mm(nc, kp, wk_sb[:, ko], xt[:, ko],
                            ko == 0, ko == KO - 1, F32R_QKV)
                    ks = rtmp.tile([HD, SCW], F32, tag="qs")
                    nc.scalar.copy(ks, kp)
                    rpk = psA.tile([HD, SCW], F32, tag="rp")
                    _mm(nc, rpk, rotT_sb[0:HD, 0:HD], ks, True, True, False)
                    t0k = rtmp.tile([HD, SCW], F32, tag="t0")
                    nc.vector.tensor_mul(t0k, ks, cosf_sb[0:HD, sl])
                    t1k = rtmp.tile([HD, SCW], F32, tag="t1")
                    nc.vector.tensor_mul(t1k, rpk, sinf_sb[0:HD, sl])
                    nc.vector.tensor_add(kTr[:, sl], t0k, t1k)
                    # v projection: [s, hd] layout (s on partitions)
                    for si in range(SCW // P):
                        vp = psA.tile([P, HD], F32, tag="vp")
                        for ko in range(KO):
                            _mm(nc, vp, xt[:, ko, si * P:(si + 1) * P],
                                wv_sb[:, ko], ko == 0, ko == KO - 1, F32R_QKV)
                        nc.scalar.copy(v_sb[:, sc * (SCW // P) + si], vp)

            # ---------------- Phase B: causal GQA attention --------------
            with (
                tc.tile_pool(name="probs", bufs=2) as prp,
                tc.tile_pool(name="sums", bufs=3) as smp,
                tc.tile_pool(name="ptb", bufs=1) as ptp,
                tc.tile_pool(name="psB", bufs=1, space="PSUM") as psB,
            ):
                for h in range(HPC):
                    for qb in range(NQB):
                        nkt = 4 * (qb + 1)
                        pts = ptp.tile([P, S // P, 512], F32R if F32R_PV else F32,
                                       tag="pts")
                        for r in range(1, 4):
                            nc.vector.memzero(pts[:, qb * 4 + r, 0:r * P])
                        for ql in range(4):
                            qi = qb * 4 + ql
                            nk = qi + 1
                            nch = (nk + 3) // 4
                            pr = prp.tile([P, S], F32R if F32R_TRANS else F32,
                                          tag="pr")
                            acc = smp.tile([P, 4], F32, tag="acc")
                            for c in range(nch):
                                w = min(512, nk * P - c * 512)
                                sp = psB.tile([P, 512], F32, tag="sp", bufs=3)
                                _mm(nc, sp[:, :w],
                                    qTa[:, h, qi * P:(qi + 1) * P],
                                    kTr[:, c * 512:c * 512 + w],
                                    True, True, F32R_SCORES)
                                if c == nch - 1:
                                    nc.vector.tensor_add(
                                        sp[:, w - P:w], sp[:, w - P:w], mask_sb)
                                nc.scalar.activation(
                                    pr[:, c * 512:c * 512 + w], sp[:, :w], Exp,
                                    scale=1.0 / math.sqrt(HD),
                                    accum_out=acc[:, c:c + 1])
                            tot = smp.tile([P, 1], F32, tag="tot")
                            nc.vector.tensor_reduce(
                                tot, acc[:, 0:nch], axis=mybir.AxisListType.X,
                                op=mybir.AluOpType.add)
                            rcp = smp.tile([P, 1], F32, tag="rcp")
                            nc.vector.reciprocal(rcp, tot)
                            nc.vector.tensor_scalar_mul(
                                pr[:, :nk * P], pr[:, :nk * P], rcp)
                            for kt in range(nk):
                                tp = psB.tile([P, P], F32R if F32R_TRANS else F32,
                                              tag="tp", bufs=2)
                                nc.tensor.transpose(
                                    tp, pr[:, kt * P:(kt + 1) * P], ident_r)
                                nc.any.tensor_copy(
                                    pts[:, kt, ql * P:(ql + 1) * P], tp)
                        op = psB.tile([HD, 512], F32, tag="op", bufs=2)
                        for kt in range(nkt):
                            _mm(nc, op, v_sb[:, kt], pts[:, kt],
                                kt == 0, kt == nkt - 1, F32R_PV)
                        nc.any.tensor_copy(
                            attnT[:, h, qb * 512:(qb + 1) * 512], op)

            # ---------------- Phase C: all-gather + Wo -------------------
            agin = dp.tile([HPC * HD, S], F32)
            agout = dp.tile([H * HD, S], F32, addr_space="Shared")
            for h in range(HPC):
                nc.sync.dma_start(agin[h * HD:(h + 1) * HD], attnT[:, h])
            nc.gpsimd.collective_compute(
                "AllGather", mybir.AluOpType.bypass,
                replica_groups=[list(range(NCORES))],
                ins=[agin.opt()], outs=[agout.opt()])

            with (
                tc.tile_pool(name="rhp", bufs=3) as rhp,
                tc.tile_pool(name="osb", bufs=2) as osb,
                tc.tile_pool(name="psC", bufs=2, space="PSUM") as psC,
            ):
                FO = H * HD // P  # 16 feature chunks
                for sc in range(S // 512):
                    sl = slice(sc * 512, (sc + 1) * 512)
                    wop = [psC.tile([P, 512], F32, tag=f"wop{c2}",
                                    name=f"wop{c2}_{sc}")
                           for c2 in range(MQ)]
                    for fo in range(FO):
                        rt = rhp.tile([P, 512], F32, tag="rt")
                        nc.sync.dma_start(
                            rt, agout[fo * P:(fo + 1) * P, sl])
                        for c2 in range(MQ):
                            _mm(nc, wop[c2], wo_sb[:, fo, c2 * P:(c2 + 1) * P],
                                rt, fo == 0, fo == FO - 1, F32R_WO)
                    for c2 in range(MQ):
                        os_t = osb.tile([P, 512], F32, tag="os")
                        nc.scalar.copy(os_t, wop[c2])
                        nc.sync.dma_start(outT_d[c2, :, sl], os_t)

    nc.compile()
    return nc


def make_in_maps(hidden_states, Wq, Wk, Wv, Wo):
    S = hidden_states.shape[1]
    xT = np.ascontiguousarray(hidden_states.reshape(S, D).T)  # [D, S]
    xTd = np.ascontiguousarray(
        xT.reshape(KO, P, S).transpose(1, 0, 2))              # [128, 16, S]

    def wslice(w, j, width):
        ws = w[:, j * width:(j + 1) * width]                  # [D, width]
        return np.ascontiguousarray(ws.reshape(KO, P, width).transpose(1, 0, 2))

    in_maps = []
    for j in range(NCORES):
        in_maps.append({
            "xT": xTd,
            "wq": wslice(Wq, j, MQ * P),
            "wk": wslice(Wk, j, HD),
            "wv": wslice(Wv, j, HD),
            "wo": wslice(Wo, j, MQ * P),
        })
    return in_maps


def assemble_output(results, S):
    out = np.empty((S, D), np.float32)
    for j in range(NCORES):
        blk = results[j]["outT"].reshape(MQ * P, S)           # [256, S]
        out[:, j * MQ * P:(j + 1) * MQ * P] = blk.T
    return out.reshape(1, S, D)


_PROG = {}
_RUN = {}
LAST = {}


class _Runner:
    """Persistent jitted SPMD executor mirroring bass2jax.run_bass_via_pjrt,
    kept alive so repeated calls skip retracing and input re-transfer."""

    def __init__(self, nc, n_cores):
        import jax
        from jax.experimental.shard_map import shard_map
        from jax.sharding import Mesh, NamedSharding, PartitionSpec

        from concourse import mybir as _mb
        from concourse.bass2jax import (_bass_exec_p, install_neuronx_cc_hook,
                                        partition_id_tensor)

        install_neuronx_cc_hook()
        self.jax = jax
        self.n_cores = n_cores
        partition_name = (nc.partition_id_tensor.name
                          if nc.partition_id_tensor else None)
        in_names, out_names, out_avals, zero_outs = [], [], [], []
        for alloc in nc.m.functions[0].allocations:
            if not isinstance(alloc, _mb.MemoryLocationSet):
                continue
            name = alloc.memorylocations[0].name
            if alloc.kind == "ExternalInput":
                if name != partition_name:
                    in_names.append(name)
            elif alloc.kind == "ExternalOutput":
                out_names.append(name)
                shape = tuple(alloc.tensor_shape)
                dtype = _mb.dt.np(alloc.dtype)
                out_avals.append(jax.core.ShapedArray(shape, dtype))
                zero_outs.append(np.zeros(shape, dtype))
        n_params = len(in_names)
        self.in_names = list(in_names)
        self.out_names = out_names
        self.out_avals = out_avals
        self.zero_outs = zero_outs
        all_in_names = in_names + out_names
        if partition_name is not None:
            all_in_names.append(partition_name)

        def _body(*args):
            operands = list(args)
            if partition_name is not None:
                operands.append(partition_id_tensor())
            outs = _bass_exec_p.bind(
                *operands, out_avals=tuple(out_avals),
                in_names=tuple(all_in_names), out_names=tuple(out_names),
                lowering_input_output_aliases=(), sim_require_finite=True,
                sim_require_nnan=True, nc=nc)
            return tuple(outs)

        devices = jax.devices()[:n_cores]
        self.mesh = Mesh(np.asarray(devices), ("core",))
        self.sharding = NamedSharding(self.mesh, PartitionSpec("core"))
        n_outs = len(out_names)
        self.fn = jax.jit(
            shard_map(_body, mesh=self.mesh,
                      in_specs=(PartitionSpec("core"),) * (n_params + n_outs),
                      out_specs=(PartitionSpec("core"),) * n_outs,
                      check_rep=False),
            donate_argnums=tuple(range(n_params, n_params + n_outs)),
            keep_unused=True)

    def put_inputs(self, in_maps):
        cat = [np.concatenate([np.asarray(m[n]) for m in in_maps], axis=0)
               for n in self.in_names]
        return [self.jax.device_put(a, self.sharding) for a in cat]

    def put_zeros(self):
        return [self.jax.device_put(
                    np.zeros((self.n_cores * z.shape[0], *z.shape[1:]), z.dtype),
                    self.sharding)
                for z in self.zero_outs]

    def run(self, dev_in, dev_zeros):
        outs = self.fn(*dev_in, *dev_zeros)
        self.jax.block_until_ready(outs)
        return outs

    def results(self, outs):
        return [
            {name: np.asarray(outs[i]).reshape(self.n_cores,
                                               *self.out_avals[i].shape)[c]
             for i, name in enumerate(self.out_names)}
            for c in range(self.n_cores)
        ]


def _get_runner(S):
    if S not in _RUN:
        if S not in _PROG:
            _PROG[S] = build_program(S)
        _RUN[S] = _Runner(_PROG[S], NCORES)
    return _RUN[S]


def kernel(hidden_states, Wq, Wk, Wv, Wo):
    import time
    hidden_states = np.asarray(hidden_states, np.float32)
    S = hidden_states.shape[1]
    r = _get_runner(S)
    in_maps = make_in_maps(hidden_states, np.asarray(Wq, np.float32),
                           np.asarray(Wk, np.float32),
                           np.asarray(Wv, np.float32),
                           np.asarray(Wo, np.float32))
    dev_in = r.put_inputs(in_maps)
    outs = r.run(dev_in, r.put_zeros())
    results = r.results(outs)
    if os.environ.get("ATTN_BENCH", "0") != "0":
        times = []
        for _ in range(int(os.environ.get("ATTN_BENCH_ITERS", "6"))):
            zs = r.put_zeros()
            t0 = time.perf_counter()
            o = r.run(dev_in, zs)
            times.append(time.perf_counter() - t0)
            del o
        LAST["bench_times"] = times
        LAST["exec_time_ns"] = int(min(times) * 1e9)
    return assemble_output(results, S)
